# revision 4
# baseline (speedup 1.0000x reference)
"""SimCLR NT-Xent contrastive loss on 8 Trainium2 cores — collective version.

Reference math (B=2048, D=256, T=0.5):
    zn = l2norm_rows(concat(z_i, z_j))          # [4096, 256]
    sim = zn @ zn.T / T                         # [4096, 4096]
    loss = mean_g [ log(sum_j exp(sim[g,j]) - exp(sim[g,g])) - sim[g, (g+B)%N] ]

Host->device traffic is the dominant cost under full_io, so each core
receives only ITS OWN 512-row slice of z, transposed to d-major fp16
([256, 512] = 0.25 MB vs the 4 MB full-z copy the old kernel shipped), plus
a tiny [128, 16] one-hot mask marking which gathered block holds its
positive pairs.  On device each core:

  1. normalizes its slice (sumsq via M=32 all-ones matmul over both
     interleaved d-halves, rsqrt as exp(-0.5*ln) on ACT, K=1 ones matmul
     broadcast, DVE multiply) -> bf16 slice, d interleaved as d = 2p + q
     (flat row-major [256, 512] <-> SBUF [128 p, 2 q, 512 n], so every DMA
     is one contiguous burst)
  2. stores the normalized slice to DRAM and AllGathers across the 8 cores
     -> the full normalized z^T [2048, 512] bf16 in 8 rank-ordered blocks
  3. sim row-block matmuls: lhsT = my slice (4 groups of 128 rows), rhs =
     gathered blocks; ACT exp(2*sim) with accum_out -> per-row rowsums
  4. positive dots: DVE scalar_tensor_tensor(-2 * mine * block_b) with
     accum_out for ALL 8 blocks -> tailP[128, 16] per-(block, d-half)
     partials; multiplying by the host mask keeps only the positive block
     (every core runs the identical program — no core-dependent indexing)
  5. log(rowsum - e^2) (the self term exp(sim[g,g]) = e^2 for normalized
     rows), reduce everything to one fp32 partial, DMA out

Host sums the 8 partials and divides by 4096.
"""

import numpy as np

import concourse.bacc as bacc
import concourse.bass as bass
import concourse.bass_isa as bass_isa
import concourse.tile as tile
from concourse import mybir

F32 = mybir.dt.float32
BF16 = mybir.dt.bfloat16
FP16 = mybir.dt.float16
AF = mybir.ActivationFunctionType
ALU = mybir.AluOpType
AXIS = mybir.AxisListType

B = 2048
D = 256
N = 2 * B            # 4096 total rows
NCORES = 8
RPC = N // NCORES    # 512 rows per core
CW = 512             # block/chunk width (one core's rows)
E2 = float(np.exp(np.float32(2.0)))   # exp(sim[g,g]) = exp(1/T)


class _Bacc(bacc.Bacc):
    """Bacc that pins the activation-table pass to the one set containing
    both Ln and Exp — the default fixpoint picks per-function sets and
    thrashes table loads into the schedule."""

    def insert_act_table_loads(self):
        from concourse.hw_specs import get_activation_tables
        import bass_rust as _bass_rust

        has_activation = any(
            isinstance(i, mybir.InstActivation)
            for b in self.main_func.blocks
            for i in b.instructions
        )
        if not has_activation:
            return
        keep = {
            mybir.ActivationFunctionType.Ln,
            mybir.ActivationFunctionType.Exp,
        }
        tables = [
            (k, v if k == "natural_log_exp_and_others" else v - keep)
            for k, v in get_activation_tables(self.m.arch).items()
        ]
        _bass_rust.insert_act_table_loads(self, tables)


def build_nc():
    nc = _Bacc(
        "TRN2", target_bir_lowering=False, debug=False, num_devices=NCORES
    )
    zs = nc.dram_tensor("zs", [D, CW], FP16, kind="ExternalInput").ap()
    pm = nc.dram_tensor("pm", [128, 16], F32, kind="ExternalInput").ap()
    out = nc.dram_tensor("out", [1, 1], F32, kind="ExternalOutput").ap()
    with tile.TileContext(nc) as tc:
        build_tile_program(tc, out, zs, pm)
    nc.compile()
    return nc


def build_tile_program(tc: tile.TileContext, out: bass.AP, zs: bass.AP, pm: bass.AP):
    nc = tc.nc
    # d-major slice viewed [p, q, n] with d = 2p + q (flat row-major pairs)
    zs_v = zs.rearrange("(p q) n -> p q n", p=128)

    with (
        tc.tile_pool(name="consts", bufs=1) as consts,
        tc.tile_pool(name="smalls", bufs=1) as smalls,
        tc.tile_pool(name="zntp", bufs=1) as zntp,
        tc.tile_pool(name="scrp", bufs=2) as scrp,
        tc.tile_pool(name="pdscrp", bufs=2) as pdscrp,
        tc.tile_pool(name="dram", bufs=1, space="DRAM") as dram,
        tc.tile_pool(name="sspsp", bufs=1, space="PSUM") as sspsp,
        tc.tile_pool(name="invp", bufs=1, space="PSUM") as invp,
        tc.tile_pool(name="simp", bufs=3, space="PSUM") as simp,
    ):
        ones32 = consts.tile([128, 32], BF16, tag="ones32")
        nc.vector.memset(ones32, 1.0)
        ones_row = consts.tile([128, 128], F32, tag="ones_row")
        nc.vector.memset(ones_row, 1.0)
        zero_col = consts.tile([128, 1], F32, tag="zero_col")
        nc.vector.memset(zero_col, 0.0)
        neg_e2 = consts.tile([128, 1], F32, tag="neg_e2")
        nc.vector.memset(neg_e2, -E2)

        # persistent small tiles
        zst = smalls.tile([128, 2, CW], FP16, tag="zst")
        sq = smalls.tile([128, 2, CW], BF16, tag="sq")
        znsb = smalls.tile([128, 2, CW], BF16, tag="znsb")
        pmt = smalls.tile([128, 16], F32, tag="pmt")
        acc16 = smalls.tile([128, 16], F32, tag="acc16")
        tailP = smalls.tile([128, 16], F32, tag="tailP")
        tailbuf = smalls.tile([128, 20], F32, tag="tailbuf")
        rowsum4 = smalls.tile([128, 4], F32, tag="rowsum4")
        tail1 = smalls.tile([128, 1], F32, tag="tail1")
        result = smalls.tile([128, 1], F32, tag="result")

        znt = zntp.tile([128, NCORES, 2, CW], BF16)  # gathered zn^T blocks

        # ---- load my slice + mask (both single contiguous bursts)
        nc.sync.dma_start(out=zst, in_=zs_v)
        nc.sync.dma_start(out=pmt, in_=pm)

        # ---- normalize my 512 rows
        zst_f = zst.rearrange("p q n -> p (q n)")
        sq_f = sq.rearrange("p q n -> p (q n)")
        nc.vector.tensor_mul(sq_f, zst_f, zst_f)
        ssps = sspsp.tile([128, CW], F32, tag="ssps")
        for q in range(2):
            nc.tensor.matmul(
                ssps[0:32, :],
                ones32,
                sq[:, q, :],
                start=(q == 0),
                stop=(q == 1),
            )
        lng = smalls.tile([1, CW], F32, tag="lng")
        invg = smalls.tile([1, CW], F32, tag="invg")
        nc.scalar.activation(lng, ssps[0:1, :], AF.Ln, bias=zero_col[0:1, :])
        nc.scalar.activation(invg, lng, AF.Exp, bias=zero_col[0:1, :], scale=-0.5)
        invrep = invp.tile([128, CW], F32, tag="invrep")
        nc.tensor.matmul(
            invrep, ones_row[0:1, :], invg, start=True, stop=True
        )
        for q in range(2):
            nc.vector.tensor_mul(znsb[:, q, :], zst[:, q, :], invrep)

        # ---- publish my normalized slice, gather everyone's
        cin = dram.tile([D, CW], BF16)
        cout = dram.tile([NCORES * D, CW], BF16, addr_space="Shared")
        cin_v = cin.rearrange("(p q) n -> p q n", p=128)
        nc.sync.dma_start(out=cin_v, in_=znsb)
        nc.gpsimd.collective_compute(
            "AllGather",
            ALU.bypass,
            replica_groups=[list(range(NCORES))],
            ins=[cin.opt()],
            outs=[cout.opt()],
        )
        cout_v = cout.rearrange("(b p q) n -> b p q n", b=NCORES, p=128)
        for b in range(NCORES):
            nc.sync.dma_start(out=znt[:, b, :, :], in_=cout_v[b])

        # ---- positive-pair dots against every block; mask selects later
        for b in range(NCORES):
            for q in range(2):
                pd_scr = pdscrp.tile([128, CW], BF16, tag="pd_scr")
                nc.vector.scalar_tensor_tensor(
                    out=pd_scr,
                    in0=znsb[:, q, :],
                    scalar=-2.0,
                    in1=znt[:, b, q, :],
                    op0=ALU.mult,
                    op1=ALU.mult,
                    accum_out=tailP[:, 2 * b + q : 2 * b + q + 1],
                )

        # ---- main sim matmuls + exp row-sum accumulation
        # column pairs (2 blocks = 1024 cols) x 4 row groups of my slice
        for cg in range(4):
            for rc in range(4):
                ps = simp.tile([128, 1024], F32, tag="ps")
                for q in range(2):
                    lhsT = znsb[:, q, rc * 128 : (rc + 1) * 128]
                    for s in range(2):
                        nc.tensor.matmul(
                            ps[:, s * CW : (s + 1) * CW],
                            lhsT,
                            znt[:, 2 * cg + s, q, :],
                            start=(q == 0),
                            stop=(q == 1),
                        )
                scr = scrp.tile([128, 1024], BF16, tag="exp_scr")
                k = cg * 4 + rc
                nc.scalar.activation(
                    scr,
                    ps,
                    AF.Exp,
                    bias=zero_col,
                    scale=2.0,
                    accum_out=acc16[:, k : k + 1],
                )

        # ---- tail: rowsums, log(neg), masked positives, total partial
        acc_v = acc16.rearrange("p (s r) -> p r s", s=4)
        nc.vector.tensor_reduce(out=rowsum4, in_=acc_v, axis=AXIS.X, op=ALU.add)
        nc.scalar.activation(tailbuf[:, 0:4], rowsum4, AF.Ln, bias=neg_e2)
        nc.vector.tensor_mul(tailbuf[:, 4:20], tailP, pmt)
        nc.vector.tensor_reduce(out=tail1, in_=tailbuf, axis=AXIS.X, op=ALU.add)
        nc.gpsimd.partition_all_reduce(
            result, tail1, channels=128, reduce_op=bass_isa.ReduceOp.add
        )
        nc.sync.dma_start(out=out, in_=result[0:1, :])


_NC_CACHE = None


def _get_nc():
    global _NC_CACHE
    if _NC_CACHE is None:
        _NC_CACHE = build_nc()
    return _NC_CACHE


def make_in_maps(z_i: np.ndarray, z_j: np.ndarray):
    z = np.concatenate(
        [np.asarray(z_i, np.float32), np.asarray(z_j, np.float32)], axis=0
    )
    in_maps = []
    for c in range(NCORES):
        zs = np.ascontiguousarray(z[c * RPC : (c + 1) * RPC].T).astype(np.float16)
        pmk = np.zeros((128, 16), np.float32)
        partner = (c + NCORES // 2) % NCORES
        pmk[:, 2 * partner : 2 * partner + 2] = 1.0
        in_maps.append({"zs": zs, "pm": pmk})
    return in_maps


_EXEC_CACHE = None


def _get_exec():
    """Jitted 8-core SPMD executable, built once and reused across calls."""
    global _EXEC_CACHE
    if _EXEC_CACHE is None:
        import jax
        from jax.experimental.shard_map import shard_map
        from jax.sharding import Mesh, PartitionSpec

        from concourse import bass2jax

        nc = _get_nc()
        bass2jax.install_neuronx_cc_hook()
        assert nc.dbg_addr is None
        part_name = (
            nc.partition_id_tensor.name if nc.partition_id_tensor else None
        )
        in_names = ["zs", "pm", "out"] + ([part_name] if part_name else [])
        out_avals = (jax.core.ShapedArray((1, 1), np.float32),)

        def _body(*args):
            operands = list(args)
            if part_name is not None:
                operands.append(bass2jax.partition_id_tensor())
            outs = bass2jax._bass_exec_p.bind(
                *operands,
                out_avals=out_avals,
                in_names=tuple(in_names),
                out_names=("out",),
                lowering_input_output_aliases=(),
                sim_require_finite=True,
                sim_require_nnan=True,
                nc=nc,
            )
            return tuple(outs)

        devices = jax.devices()[:NCORES]
        mesh = Mesh(np.asarray(devices), ("core",))
        sharded = jax.jit(
            shard_map(
                _body,
                mesh=mesh,
                in_specs=(PartitionSpec("core"),) * 3,
                out_specs=(PartitionSpec("core"),),
                check_rep=False,
            ),
            donate_argnums=(2,),
            keep_unused=True,
        )
        _EXEC_CACHE = sharded
    return _EXEC_CACHE


def run_cores(in_maps):
    """Run the SPMD kernel; returns the 8 per-core [1,1] partials."""
    sharded = _get_exec()
    concat_zs = np.concatenate([m["zs"] for m in in_maps], axis=0)
    concat_pm = np.concatenate([m["pm"] for m in in_maps], axis=0)
    zeros = np.zeros((NCORES, 1), np.float32)
    (out,) = sharded(concat_zs, concat_pm, zeros)
    return np.asarray(out)  # [NCORES, 1]


def kernel(z_i: np.ndarray, z_j: np.ndarray) -> np.ndarray:
    partials = run_cores(make_in_maps(z_i, z_j))
    return np.float32(float(partials.sum()) / N)
